# revision 2
# baseline (speedup 1.0000x reference)
"""CTC loss (keras ctc_batch_cost semantics) on 8 Trainium2 NeuronCores.

Data parallel: 32 examples per core. The device input is the MINIMAL
label-gathered probability stream in fp8_e4m3:

    g[j, t, b] = ZQ * (y_pred[b, t, idx_j] + EPS),  idx = [48 labels | blank]

0.82 MB per core (vs 8 MB for precomputed composite coefficients, 17 MB for
raw y_pred) -- the real-HW execution is input-DMA-bound, so bytes are the
metric that matters.

On device:
  1. one fp8 x fp8 matmul per 512-column chunk expands g (49 rows) to the
     97 extended CTC states:  Q = W_E^T @ g  (W_E is the fixed 0/1 map
     s odd -> label (s-1)/2, s even -> blank).  PSUM -> SBUF copies emit the
     bf16 [q | r] stream, with r = mask_shift2 * q folded in by the vector
     engine (per-example skip mask, constant over t).
  2. the sequential alpha recurrence (states on partitions, [97, batch]):
         z_t = W1 @ (q_t * z_{t-1}) + W2 @ (r_t * z_{t-1})
     one vector multiply + two PSUM-accumulating matmuls per step, two
     16-example groups ping-ponging the engines.  Every 32 steps the state
     is renormalized: cs = sum(u); u,v *= 1/cs, with the partition broadcast
     of 1/cs done by a K=1 matmul (no GPSIMD).
  3. loss = -(log(u_T[95]+u_T[96]) + sum_j log(cs_j) - T*log(ZQ)).

NOTE on DMA structure: this walrus build lowers DMA/memset to pseudo-DMA
instructions that accept at most ONE sync-wait command, so the program keeps
all loads write-once/dependency-free and budgets < 8 DMA-lowered
instructions before the single (dependency-carrying) loss store.
"""
import os
import sys
import numpy as np

for _p in ("/opt/trn_rl_repo", "/root/.axon_site/_ro/trn_rl_repo"):
    if os.path.isdir(_p) and _p not in sys.path:
        sys.path.insert(0, _p)

import ml_dtypes  # noqa: E402
import concourse.bass as bass  # noqa: E402
import concourse.bacc as bacc  # noqa: E402
import concourse.mybir as mybir  # noqa: E402
import concourse.tile as tile  # noqa: E402
from concourse.bass_utils import run_bass_kernel_spmd  # noqa: E402

BF = ml_dtypes.bfloat16
F32 = np.float32
FP8 = ml_dtypes.float8_e4m3

B, T, L, C = 256, 512, 48, 512
S = 2 * L + 1          # 97
NJ = L + 1             # 49 gather targets: 48 labels + blank
BLANK = C - 1
EPS = 1e-7
ZQ = 512.0             # per-step scale folded into the coefficients
NCORES = 8
BPC = B // NCORES      # 32 examples per core
TN = T * BPC           # 16384 g columns (t-major, example-fastest)
RESC = 32              # rescale interval (steps); cs must stay < 2^64 for Ln
CHT = 16               # expansion chunk: 16 timesteps = 512 columns
NCHUNK = T // CHT      # 32 chunks

# aux column layout
_C_W1 = 0
_C_W2 = 97
_C_ONES = 194
_C_SEL = 195
_C_E01 = 196
_C_WBC = 197
_C_MD2 = 294
NCOL = _C_MD2 + BPC    # 326

# s -> gather row (fixed, example-independent)
_JMAP = np.where(np.arange(S) % 2 == 1, (np.arange(S) - 1) // 2, L)


# ---------------------------------------------------------------------------
# host-side precompute
# ---------------------------------------------------------------------------

def _host_we():
    we = np.zeros((NJ, S), dtype=F32)
    we[_JMAP, np.arange(S)] = 1.0
    return we.astype(FP8)


def _host_aux_const():
    aux = np.zeros((S, _C_MD2), dtype=F32)
    ss = np.arange(S)
    aux[ss, _C_W1 + ss] = 1.0                    # W1: k == s
    aux[ss[1:] - 1, _C_W1 + ss[1:]] = 1.0        # W1: k == s-1
    aux[ss[2:] - 2, _C_W2 + ss[2:]] = 1.0        # W2: k == s-2
    aux[:, _C_ONES] = 1.0
    aux[S - 2:S, _C_SEL] = 1.0                   # final states 95, 96
    aux[0:2, _C_E01] = 1.0                       # alpha0 states 0, 1
    aux[0, _C_WBC:_C_WBC + S] = 1.0              # K=1 broadcast row
    return aux.astype(BF)


def make_in_maps(y_true, y_pred):
    lab = np.asarray(y_true).astype(np.int64)          # [B, L]
    y = np.asarray(y_pred, dtype=F32)                  # [B, T, C]

    idx = np.concatenate(
        [lab, np.full((B, 1), BLANK, np.int64)], axis=1)   # [B, NJ]
    g = np.take_along_axis(y, idx[:, None, :], axis=2)     # [B, T, NJ]
    g8 = ((g + EPS) * ZQ).astype(FP8)

    ext = np.full((B, S), BLANK, dtype=np.int64)
    ext[:, 1::2] = lab
    m = np.zeros((B, S), dtype=F32)
    m[:, 1] = 1.0
    odd = np.arange(3, S, 2)
    m[:, odd] = (ext[:, odd] != ext[:, odd - 2]).astype(F32)
    md2 = np.zeros((B, S), dtype=F32)
    md2[:, :S - 2] = m[:, 2:]

    we = _host_we()
    aux_const = _host_aux_const()
    in_maps = []
    for core in range(NCORES):
        sl = slice(core * BPC, (core + 1) * BPC)
        gc = np.ascontiguousarray(
            g8[sl].transpose(2, 1, 0)).reshape(NJ, TN)     # [NJ, T*n]
        g_in = np.concatenate([gc, we], axis=1)            # [NJ, TN+S] fp8
        aux = np.concatenate(
            [aux_const, md2[sl].T.astype(BF)], axis=1)     # [S, NCOL] bf16
        in_maps.append({"g": g_in, "aux": aux})
    return in_maps


# ---------------------------------------------------------------------------
# device program
# ---------------------------------------------------------------------------

def build_bass(n_ex=BPC, Tt=T):
    dtb = mybir.dt.bfloat16
    dtf = mybir.dt.float32
    dt8 = mybir.dt.float8e4
    resc_ts = [t for t in range(RESC, Tt - 1, RESC)]   # 32..480 -> 15
    ncs = len(resc_ts) + 1
    tn = Tt * n_ex

    nc = bacc.Bacc()
    g_d = nc.dram_tensor("g", [NJ, tn + S], dt8, kind="ExternalInput")
    aux_d = nc.dram_tensor("aux", [S, NCOL], dtb, kind="ExternalInput")
    loss_d = nc.dram_tensor("loss", [n_ex, 1], dtf, kind="ExternalOutput")

    with tile.TileContext(nc) as tc:
        with (
            tc.tile_pool(name="persist", bufs=1) as persist,
            tc.tile_pool(name="uv", bufs=2) as uv_pool,
        ):
            gt = persist.tile([NJ, tn + S], dt8, tag="gt")
            aux_t = persist.tile([S, NCOL], dtb, tag="aux")
            qr = persist.tile([S, Tt, 2, n_ex], dtb, tag="qr")
            z0t = persist.tile([S, n_ex], dtf, tag="z0t")
            cbuf = persist.tile([1, ncs, n_ex], dtf, tag="cbuf")
            logbuf = persist.tile([1, ncs, n_ex], dtf, tag="logbuf")
            rscale = persist.tile([1, n_ex], dtb, tag="rscale")
            llsum = persist.tile([1, n_ex], dtf, tag="llsum")
            lossb = persist.tile([1, n_ex], dtf, tag="lossb")

            nc.sync.dma_start(gt[:], g_d[:])
            nc.scalar.dma_start(aux_t[:], aux_d[:])

            we_ap = gt[:, tn:tn + S]                      # [NJ, S] fp8
            w1 = aux_t[:, _C_W1:_C_W1 + S]
            w2 = aux_t[:, _C_W2:_C_W2 + S]
            ones_col = aux_t[:, _C_ONES:_C_ONES + 1]
            sel_col = aux_t[:, _C_SEL:_C_SEL + 1]
            e01_col = aux_t[:, _C_E01:_C_E01 + 1]
            wbc_row = aux_t[0:1, _C_WBC:_C_WBC + S]       # [1, S]
            md2_ap = aux_t[:, _C_MD2:_C_MD2 + n_ex]       # [S, n]

            # ---- expansion: g (49 rows) -> bf16 [q|r] stream (97 states)
            with tc.tile_pool(name="exp", bufs=2, space="PSUM") as expP:
                for ci in range(NCHUNK):
                    tlo = ci * CHT
                    ex = expP.tile([S, CHT, n_ex], dtf, tag="ex",
                                   name=f"ex{ci}")
                    nc.tensor.matmul(
                        ex[:], we_ap,
                        gt[:, tlo * n_ex:(tlo + CHT) * n_ex],
                        start=True, stop=True)
                    nc.scalar.copy(qr[:, tlo:tlo + CHT, 0, :], ex[:])
                    nc.vector.tensor_tensor(
                        qr[:, tlo:tlo + CHT, 1, :], ex[:],
                        md2_ap.unsqueeze(1).broadcast_to([S, CHT, n_ex]),
                        mybir.AluOpType.mult)

            # ---- alpha_0 on states 0,1
            nc.vector.tensor_tensor(
                z0t[:], qr[:, 0, 0, :],
                e01_col.broadcast_to([S, n_ex]),
                mybir.AluOpType.mult)

            # ---- recurrence
            NG = 2
            gsz = n_ex // NG
            gsl = [slice(gg * gsz, (gg + 1) * gsz) for gg in range(NG)]
            with (
                tc.tile_pool(name="zp", bufs=2, space="PSUM") as zP,
                tc.tile_pool(name="csp", bufs=1, space="PSUM") as csP,
                tc.tile_pool(name="rbp", bufs=1, space="PSUM") as rbP,
            ):
                yt = [[uv_pool.tile([S, 2, gsz], dtb, tag=f"y{g}{p}",
                                    name=f"y{g}{p}") for p in range(2)]
                      for g in range(NG)]
                al_prev = [None] * NG
                ylast = [None] * NG
                for t in range(1, Tt):
                    for g in range(NG):
                        y = yt[g][t % 2]
                        if t == 1:
                            src_ap = z0t[:, gsl[g]].unsqueeze(1)\
                                .broadcast_to([S, 2, gsz])
                        else:
                            src_ap = al_prev[g][:].unsqueeze(1)\
                                .broadcast_to([S, 2, gsz])
                        nc.vector.tensor_tensor(
                            y[:], src_ap, qr[:, t, :, gsl[g]],
                            mybir.AluOpType.mult)
                        if t in resc_ts:
                            j = resc_ts.index(t)
                            cs = csP.tile([1, gsz], dtf, tag=f"cs{g}",
                                          name=f"cs_{t}_{g}")
                            nc.tensor.matmul(cs[:], ones_col, y[:, 0, :],
                                             start=True, stop=True)
                            with nc.allow_low_precision(
                                    reason="1/cs broadcast via bf16 matmul; "
                                    "log uses the f32 cs"):
                                nc.vector.reciprocal(rscale[:, gsl[g]],
                                                     cs[:])
                            nc.scalar.copy(cbuf[:, j, gsl[g]], cs[:])
                            rb = rbP.tile([S, gsz], dtf, tag=f"rb{g}",
                                          name=f"rb_{t}_{g}")
                            nc.tensor.matmul(rb[:], wbc_row,
                                             rscale[:, gsl[g]],
                                             start=True, stop=True)
                            nc.vector.tensor_tensor(
                                y[:], y[:],
                                rb[:].unsqueeze(1).broadcast_to([S, 2, gsz]),
                                mybir.AluOpType.mult)
                        if t == Tt - 1:
                            ylast[g] = y
                        else:
                            al = zP.tile([S, gsz], dtf, tag=f"z{g}",
                                         name=f"al_{t}_{g}")
                            nc.tensor.matmul(al[:], w1, y[:, 0, :],
                                             start=True, stop=False)
                            nc.tensor.matmul(al[:], w2, y[:, 1, :],
                                             start=False, stop=True)
                            al_prev[g] = al

                # ---- final: alphaT = u_T; fin = u_T[95] + u_T[96]
                for g in range(NG):
                    fin = csP.tile([1, gsz], dtf, tag=f"cs{g}",
                                   name=f"fin{g}")
                    nc.tensor.matmul(fin[:], sel_col, ylast[g][:, 0, :],
                                     start=True, stop=True)
                    nc.scalar.copy(cbuf[:, ncs - 1, gsl[g]], fin[:])

            nc.scalar.activation(logbuf[:], cbuf[:],
                                 mybir.ActivationFunctionType.Ln)
            nc.vector.tensor_reduce(
                llsum[:], logbuf[:].rearrange("p j b -> p b j"),
                mybir.AxisListType.X, mybir.AluOpType.add)
            for _ in range(2):
                nc.scalar.activation(lossb[:], llsum[:],
                                     mybir.ActivationFunctionType.Copy,
                                     bias=float(Tt * np.log(ZQ)), scale=-1.0)
            nc.sync.dma_start(loss_d[:, 0].unsqueeze(0), lossb[0:1, :])
    nc.compile()
    return nc


# ---------------------------------------------------------------------------
# entry point
# ---------------------------------------------------------------------------

_CACHE = {}


def _get_nc():
    if "nc" not in _CACHE:
        _CACHE["nc"] = build_bass()
    return _CACHE["nc"]


def kernel(y_true, y_pred):
    nc = _get_nc()
    in_maps = make_in_maps(y_true, y_pred)
    res = run_bass_kernel_spmd(nc, in_maps, list(range(NCORES)))
    out = np.concatenate([res.results[c]["loss"] for c in range(NCORES)],
                         axis=0)
    return out.astype(F32)


# revision 3
# speedup vs baseline: 1.5731x; 1.5731x over previous
"""CTC loss (keras ctc_batch_cost semantics) on 8 Trainium2 NeuronCores.

Data parallel: 32 examples per core. The device input is the MINIMAL
label-gathered probability stream in fp8_e4m3:

    g[j, t, b] = ZQ * (y_pred[b, t, idx_j] + EPS),  idx = [48 labels | blank]

0.82 MB per core (vs 8 MB for precomputed composite coefficients, 17 MB for
raw y_pred) -- the real-HW execution is input-DMA-bound, so bytes are the
metric that matters.

On device:
  1. one fp8 x fp8 matmul per 512-column chunk expands g (49 rows) to the
     97 extended CTC states:  Q = W_E^T @ g  (W_E is the fixed 0/1 map
     s odd -> label (s-1)/2, s even -> blank).  PSUM -> SBUF copies emit the
     bf16 [q | r] stream, with r = mask_shift2 * q folded in by the vector
     engine (per-example skip mask, constant over t).
  2. the sequential alpha recurrence (states on partitions, [97, batch]):
         z_t = W1 @ (q_t * z_{t-1}) + W2 @ (r_t * z_{t-1})
     one vector multiply + two PSUM-accumulating matmuls per step, two
     16-example groups ping-ponging the engines.  Every 32 steps the state
     is renormalized: cs = sum(u); u,v *= 1/cs, with the partition broadcast
     of 1/cs done by a K=1 matmul (no GPSIMD).
  3. loss = -(log(u_T[95]+u_T[96]) + sum_j log(cs_j) - T*log(ZQ)).

NOTE on DMA structure: this walrus build lowers DMA/memset to pseudo-DMA
instructions that accept at most ONE sync-wait command, so the program keeps
all loads write-once/dependency-free and budgets < 8 DMA-lowered
instructions before the single (dependency-carrying) loss store.
"""
import os
import sys
import numpy as np

for _p in ("/opt/trn_rl_repo", "/root/.axon_site/_ro/trn_rl_repo"):
    if os.path.isdir(_p) and _p not in sys.path:
        sys.path.insert(0, _p)

import ml_dtypes  # noqa: E402
import concourse.bass as bass  # noqa: E402
import concourse.bacc as bacc  # noqa: E402
import concourse.mybir as mybir  # noqa: E402
import concourse.tile as tile  # noqa: E402
from concourse.bass_utils import run_bass_kernel_spmd  # noqa: E402

BF = ml_dtypes.bfloat16
F32 = np.float32
FP8 = ml_dtypes.float8_e4m3

B, T, L, C = 256, 512, 48, 512
S = 2 * L + 1          # 97
NJ = L + 1             # 49 gather targets: 48 labels + blank
BLANK = C - 1
EPS = 1e-7
ZQ = 512.0             # per-step scale folded into the coefficients
NCORES = 8
BPC = B // NCORES      # 32 examples per core
TN = T * BPC           # 16384 g columns (t-major, example-fastest)
RESC = 32              # rescale interval (steps); cs must stay < 2^64 for Ln
CHT = 16               # expansion chunk: 16 timesteps = 512 columns
NCHUNK = T // CHT      # 32 chunks

# aux column layout
_C_W1 = 0
_C_W2 = 97
_C_ONES = 194
_C_SEL = 195
_C_E01 = 196
_C_WBC = 197
_C_MD2 = 294
NCOL = _C_MD2 + BPC    # 326

# s -> gather row (fixed, example-independent)
_JMAP = np.where(np.arange(S) % 2 == 1, (np.arange(S) - 1) // 2, L)


# ---------------------------------------------------------------------------
# host-side precompute
# ---------------------------------------------------------------------------

def _host_we():
    we = np.zeros((NJ, S), dtype=F32)
    we[_JMAP, np.arange(S)] = 1.0
    return we.astype(FP8)


def _host_aux_const():
    aux = np.zeros((S, _C_MD2), dtype=F32)
    ss = np.arange(S)
    aux[ss, _C_W1 + ss] = 1.0                    # W1: k == s
    aux[ss[1:] - 1, _C_W1 + ss[1:]] = 1.0        # W1: k == s-1
    aux[ss[2:] - 2, _C_W2 + ss[2:]] = 1.0        # W2: k == s-2
    aux[:, _C_ONES] = 1.0
    aux[S - 2:S, _C_SEL] = 1.0                   # final states 95, 96
    aux[0:2, _C_E01] = 1.0                       # alpha0 states 0, 1
    aux[0, _C_WBC:_C_WBC + S] = 1.0              # K=1 broadcast row
    return aux.astype(BF)


def make_in_maps(y_true, y_pred):
    lab = np.asarray(y_true).astype(np.int64)          # [B, L]
    y = np.asarray(y_pred, dtype=F32)                  # [B, T, C]

    idx = np.concatenate(
        [lab, np.full((B, 1), BLANK, np.int64)], axis=1)   # [B, NJ]
    g = np.take_along_axis(y, idx[:, None, :], axis=2)     # [B, T, NJ]
    g8 = ((g + EPS) * ZQ).astype(FP8)

    ext = np.full((B, S), BLANK, dtype=np.int64)
    ext[:, 1::2] = lab
    m = np.zeros((B, S), dtype=F32)
    m[:, 1] = 1.0
    odd = np.arange(3, S, 2)
    m[:, odd] = (ext[:, odd] != ext[:, odd - 2]).astype(F32)
    md2 = np.zeros((B, S), dtype=F32)
    md2[:, :S - 2] = m[:, 2:]

    we = _host_we()
    aux_const = _host_aux_const()
    in_maps = []
    for core in range(NCORES):
        sl = slice(core * BPC, (core + 1) * BPC)
        gc = np.ascontiguousarray(
            g8[sl].transpose(2, 1, 0)).reshape(NJ, TN)     # [NJ, T*n]
        g_in = np.concatenate([we, gc], axis=1)            # [NJ, S+TN] fp8
        aux = np.concatenate(
            [aux_const, md2[sl].T.astype(BF)], axis=1)     # [S, NCOL] bf16
        in_maps.append({"g": g_in, "aux": aux})
    return in_maps


# ---------------------------------------------------------------------------
# device program
# ---------------------------------------------------------------------------

def build_bass(n_ex=BPC, Tt=T):
    dtb = mybir.dt.bfloat16
    dtf = mybir.dt.float32
    dt8 = mybir.dt.float8e4
    resc_ts = [t for t in range(RESC, Tt - 1, RESC)]   # 32..480 -> 15
    ncs = len(resc_ts) + 1
    tn = Tt * n_ex

    nc = bacc.Bacc()
    g_d = nc.dram_tensor("g", [NJ, S + tn], dt8, kind="ExternalInput")
    aux_d = nc.dram_tensor("aux", [S, NCOL], dtb, kind="ExternalInput")
    loss_d = nc.dram_tensor("loss", [n_ex, 1], dtf, kind="ExternalOutput")

    with tile.TileContext(nc) as tc:
        with (
            tc.tile_pool(name="persist", bufs=1) as persist,
            tc.tile_pool(name="uv", bufs=2) as uv_pool,
        ):
            gt = persist.tile([NJ, S + tn], dt8, tag="gt")
            aux_t = persist.tile([S, NCOL], dtb, tag="aux")
            qr = persist.tile([S, Tt, 2, n_ex], dtb, tag="qr")
            z0t = persist.tile([S, n_ex], dtf, tag="z0t")
            cbuf = persist.tile([1, ncs, n_ex], dtf, tag="cbuf")
            logbuf = persist.tile([1, ncs, n_ex], dtf, tag="logbuf")
            rscale = persist.tile([1, n_ex], dtb, tag="rscale")
            llsum = persist.tile([1, n_ex], dtf, tag="llsum")
            lossb = persist.tile([1, n_ex], dtf, tag="lossb")

            # 4 chunked loads alternating HWDGE queues: the expansion
            # can start on quarter 0 while later quarters stream in.
            qtr = tn // 4
            for qi in range(4):
                lo = S + qi * qtr if qi else 0
                hi = S + (qi + 1) * qtr
                eng = nc.sync if qi % 2 == 0 else nc.scalar
                eng.dma_start(gt[:, lo:hi], g_d[:, lo:hi])
            nc.scalar.dma_start(aux_t[:], aux_d[:])

            we_ap = gt[:, 0:S]                            # [NJ, S] fp8
            w1 = aux_t[:, _C_W1:_C_W1 + S]
            w2 = aux_t[:, _C_W2:_C_W2 + S]
            ones_col = aux_t[:, _C_ONES:_C_ONES + 1]
            sel_col = aux_t[:, _C_SEL:_C_SEL + 1]
            e01_col = aux_t[:, _C_E01:_C_E01 + 1]
            wbc_row = aux_t[0:1, _C_WBC:_C_WBC + S]       # [1, S]
            md2_ap = aux_t[:, _C_MD2:_C_MD2 + n_ex]       # [S, n]

            # ---- expansion: g (49 rows) -> bf16 [q|r] stream (97 states)
            with tc.tile_pool(name="exp", bufs=2, space="PSUM") as expP:
                for ci in range(NCHUNK):
                    tlo = ci * CHT
                    ex = expP.tile([S, CHT, n_ex], dtf, tag="ex",
                                   name=f"ex{ci}")
                    nc.tensor.matmul(
                        ex[:], we_ap,
                        gt[:, S + tlo * n_ex:S + (tlo + CHT) * n_ex],
                        start=True, stop=True)
                    nc.scalar.copy(qr[:, tlo:tlo + CHT, 0, :], ex[:])
                    nc.vector.tensor_tensor(
                        qr[:, tlo:tlo + CHT, 1, :], ex[:],
                        md2_ap.unsqueeze(1).broadcast_to([S, CHT, n_ex]),
                        mybir.AluOpType.mult)

            # ---- alpha_0 on states 0,1
            nc.vector.tensor_tensor(
                z0t[:], qr[:, 0, 0, :],
                e01_col.broadcast_to([S, n_ex]),
                mybir.AluOpType.mult)

            # ---- recurrence
            NG = 2
            gsz = n_ex // NG
            gsl = [slice(gg * gsz, (gg + 1) * gsz) for gg in range(NG)]
            with (
                tc.tile_pool(name="zp", bufs=2, space="PSUM") as zP,
                tc.tile_pool(name="csp", bufs=1, space="PSUM") as csP,
                tc.tile_pool(name="rbp", bufs=1, space="PSUM") as rbP,
            ):
                yt = [[uv_pool.tile([S, 2, gsz], dtb, tag=f"y{g}{p}",
                                    name=f"y{g}{p}") for p in range(2)]
                      for g in range(NG)]
                al_prev = [None] * NG
                ylast = [None] * NG
                for t in range(1, Tt):
                    for g in range(NG):
                        y = yt[g][t % 2]
                        if t == 1:
                            src_ap = z0t[:, gsl[g]].unsqueeze(1)\
                                .broadcast_to([S, 2, gsz])
                        else:
                            src_ap = al_prev[g][:].unsqueeze(1)\
                                .broadcast_to([S, 2, gsz])
                        nc.vector.tensor_tensor(
                            y[:], src_ap, qr[:, t, :, gsl[g]],
                            mybir.AluOpType.mult)
                        if t in resc_ts:
                            j = resc_ts.index(t)
                            cs = csP.tile([1, gsz], dtf, tag=f"cs{g}",
                                          name=f"cs_{t}_{g}")
                            nc.tensor.matmul(cs[:], ones_col, y[:, 0, :],
                                             start=True, stop=True)
                            with nc.allow_low_precision(
                                    reason="1/cs broadcast via bf16 matmul; "
                                    "log uses the f32 cs"):
                                nc.vector.reciprocal(rscale[:, gsl[g]],
                                                     cs[:])
                            nc.scalar.copy(cbuf[:, j, gsl[g]], cs[:])
                            rb = rbP.tile([S, gsz], dtf, tag=f"rb{g}",
                                          name=f"rb_{t}_{g}")
                            nc.tensor.matmul(rb[:], wbc_row,
                                             rscale[:, gsl[g]],
                                             start=True, stop=True)
                            nc.vector.tensor_tensor(
                                y[:], y[:],
                                rb[:].unsqueeze(1).broadcast_to([S, 2, gsz]),
                                mybir.AluOpType.mult)
                        if t == Tt - 1:
                            ylast[g] = y
                        else:
                            al = zP.tile([S, gsz], dtf, tag=f"z{g}",
                                         name=f"al_{t}_{g}")
                            nc.tensor.matmul(al[:], w1, y[:, 0, :],
                                             start=True, stop=False)
                            nc.tensor.matmul(al[:], w2, y[:, 1, :],
                                             start=False, stop=True)
                            al_prev[g] = al

                # ---- final: alphaT = u_T; fin = u_T[95] + u_T[96]
                for g in range(NG):
                    fin = csP.tile([1, gsz], dtf, tag=f"cs{g}",
                                   name=f"fin{g}")
                    nc.tensor.matmul(fin[:], sel_col, ylast[g][:, 0, :],
                                     start=True, stop=True)
                    nc.scalar.copy(cbuf[:, ncs - 1, gsl[g]], fin[:])

            nc.scalar.activation(logbuf[:], cbuf[:],
                                 mybir.ActivationFunctionType.Ln)
            nc.vector.tensor_reduce(
                llsum[:], logbuf[:].rearrange("p j b -> p b j"),
                mybir.AxisListType.X, mybir.AluOpType.add)
            for _ in range(2):
                nc.scalar.activation(lossb[:], llsum[:],
                                     mybir.ActivationFunctionType.Copy,
                                     bias=float(Tt * np.log(ZQ)), scale=-1.0)
            nc.sync.dma_start(loss_d[:, 0].unsqueeze(0), lossb[0:1, :])
    nc.compile()
    return nc


# ---------------------------------------------------------------------------
# entry point
# ---------------------------------------------------------------------------

_CACHE = {}


def _get_nc():
    if "nc" not in _CACHE:
        _CACHE["nc"] = build_bass()
    return _CACHE["nc"]


def kernel(y_true, y_pred):
    nc = _get_nc()
    in_maps = make_in_maps(y_true, y_pred)
    res = run_bass_kernel_spmd(nc, in_maps, list(range(NCORES)))
    out = np.concatenate([res.results[c]["loss"] for c in range(NCORES)],
                         axis=0)
    return out.astype(F32)
